# revision 1
# baseline (speedup 1.0000x reference)
"""Trainium2 kernel for nn_Activation1d (BigVGAN up->SnakeBeta->down), 8-core SPMD.

Math (per row; a=exp(alpha), invb=1/(exp(beta)+1e-9) per channel):
    out = D @ (U@x + invb * sin^2(a * U@x))
where U (2x upsample) and D (2x downsample) are narrow-banded Toeplitz
operators with replicate-pad boundaries.

On-chip realization (shifted-window blocks, L=116 outputs per block, one
128-wide input window each -- bands are only +-6 wide so prev/main/next
halo matmuls are unnecessary):
    xut   = x window, t-major, via transposing DMA (HBM -> SBUF xbar)   [fp16]
    xat2  = xut * (a/pi)  (broadcast TT)                                [fp16]
    r01   = [U0 @ xat2 | U1 @ xat2]   (both phases, PSUM fp32)
    m01   = mod(r01 + 8, 2)           (range reduction; +8 keeps the
                                       argument positive so C-fmod ==
                                       python-mod)                      [fp16]
    s01   = Sin(pi*m01 - pi) = sin(pi * a*y/pi) = sin(a*y)              [bf16]
    q01   = s01 * (s01 * invb) = invb * sin^2(a*y)                      [bf16]
    P     = G @ xut + D0 @ q0 + D1 @ q1    (single PSUM accumulation,
                                            G = D@U probed in float64)
    out   = PE-transpose(P as bf16) -> og supertile -> HBM (bf16, host
            upconverts to fp32)

Weight variants first/int/last fold the replicate padding; probed
numerically from a pure-numpy port of the reference on an identity batch.
"""
import math
import numpy as np
import ml_dtypes

import concourse.bass as bass
import concourse.bacc as bacc
import concourse.mybir as mybir
from concourse.tile import TileContext
from concourse.bass_utils import run_bass_kernel_spmd

F32 = mybir.dt.float32
F16 = mybir.dt.float16
BF16 = mybir.dt.bfloat16

B_, C_, T_ = 16, 256, 8192
K_ = 12
NCORES = 8
ROWS = (B_ * C_) // NCORES            # 512
L_ = 116
NB = 71                               # 70 blocks of 116 + tail of 72
GRP = 8
NGRP = (NB + GRP - 1) // GRP          # 9 (last group has 7 blocks)
TP = 512                              # host probe domain

MAGIC16 = 3072.0   # 1.5 * 2**11: fp16 spacing 2.0 -> store rounds to even ints


# ---------------------------------------------------------------- host math
def _np_up_linop(x, up_filter):
    n, t = x.shape
    pad = K_ // 2 - 1
    pad_left = pad * 2 + (K_ - 2) // 2
    xp = np.concatenate([np.repeat(x[:, :1], pad, 1), x,
                         np.repeat(x[:, -1:], pad, 1)], axis=1)
    L = t + 2 * pad
    xd = np.zeros((n, 2 * L - 1), dtype=np.float32)
    xd[:, ::2] = xp
    yf = np.zeros((n, 2 * L - 1 + K_ - 1), dtype=np.float32)
    for m in range(K_):
        yf[:, m:m + 2 * L - 1] += np.float32(up_filter[m]) * xd
    return (2.0 * yf[:, pad_left:pad_left + 2 * t]).astype(np.float32)


def _np_down_linop(z, down_filter):
    n, t2 = z.shape
    dpl, dpr = K_ // 2 - 1, K_ // 2
    zp = np.concatenate([np.repeat(z[:, :1], dpl, 1), z,
                         np.repeat(z[:, -1:], dpr, 1)], axis=1)
    t = t2 // 2
    out = np.zeros((n, t), dtype=np.float32)
    for j in range(K_):
        out += np.float32(down_filter[j]) * zp[:, j:j + 2 * t - 1:2]
    return out


def block_params(b):
    """(out_start, window_start, out_len) for block b."""
    if b == NB - 1:
        return 116 * b, T_ - 128, T_ - 116 * b
    return 116 * b, max(0, 116 * b - 6), 116


def _variant_anchor(variant):
    if variant == 'first':
        return 0, 0, 116
    if variant == 'int':
        return 192, 186, 116
    return TP - 72, TP - 128, 72


_VARIANTS = ('first', 'int', 'last')
_UOPS = ('U0', 'U1', 'G')             # fp16 weight block order
_DOPS = ('D0', 'D1')                  # bf16 weight block order


def build_weights(up_filter, down_filter):
    I = np.eye(TP, dtype=np.float32)
    Ufull = _np_up_linop(I, up_filter).T
    U0p, U1p = Ufull[0::2, :], Ufull[1::2, :]
    I2 = np.eye(2 * TP, dtype=np.float32)
    Dfull = _np_down_linop(I2, down_filter).T
    D0p, D1p = Dfull[:, 0::2], Dfull[:, 1::2]
    Gp = (Dfull.astype(np.float64) @ Ufull.astype(np.float64)).astype(np.float32)

    Ws = {}
    o = np.arange(128)
    p = np.arange(128)
    for variant in _VARIANTS:
        s, w, L = _variant_anchor(variant)
        c_pos = s - 3 + o
        x_pos = w + p
        cv = (c_pos >= 0) & (c_pos < TP)
        xv = (x_pos >= 0) & (x_pos < TP)
        for name, P in (('U0', U0p), ('U1', U1p)):
            W = np.zeros((128, 128), np.float32)
            W[np.ix_(xv, cv)] = P[np.ix_(c_pos[cv], x_pos[xv])].T
            Ws[(name, variant)] = W
        out_pos = s + o
        ov = (o < L) & (out_pos < TP)
        for name, P in (('D0', D0p), ('D1', D1p)):
            W = np.zeros((128, 128), np.float32)
            W[np.ix_(cv, ov)] = P[np.ix_(out_pos[ov], c_pos[cv])].T
            Ws[(name, variant)] = W
        W = np.zeros((128, 128), np.float32)
        W[np.ix_(xv, ov)] = Gp[np.ix_(out_pos[ov], x_pos[xv])].T
        Ws[('G', variant)] = W
    return Ws


def host_consts(alpha, beta, up_filter, down_filter):
    alpha = np.asarray(alpha, dtype=np.float32)
    beta = np.asarray(beta, dtype=np.float32)
    Ws = build_weights(np.asarray(up_filter, np.float32),
                       np.asarray(down_filter, np.float32))
    wu = np.concatenate([Ws[(op, v)] for op in _UOPS for v in _VARIANTS],
                        axis=1).astype(np.float16)          # [128, 9*128]
    wd = np.concatenate([Ws[(op, v)] for op in _DOPS for v in _VARIANTS],
                        axis=1).astype(ml_dtypes.bfloat16)  # [128, 6*128]

    arow = np.exp(np.tile(alpha, ROWS // C_)).astype(np.float32)
    invb = (1.0 / (np.exp(np.tile(beta, ROWS // C_)) + 1e-9)).astype(np.float32)
    a2 = (arow / np.float32(math.pi)).astype(np.float32)
    a2t2 = np.broadcast_to(a2[None, :], (128, ROWS)).astype(np.float16)
    invbt = np.broadcast_to(np.tile(invb, 2)[None, :],
                            (128, 2 * ROWS)).astype(ml_dtypes.bfloat16)
    identb = np.eye(128, dtype=np.float32).astype(ml_dtypes.bfloat16)
    return dict(wu=wu, wd=wd, a2t2=a2t2, invbt=invbt, identb=identb)


# ---------------------------------------------------------------- bass kernel
def _build_bass():
    nc = bacc.Bacc()
    xs = nc.dram_tensor("xs", [ROWS, T_], F16, kind="ExternalInput")
    wu_d = nc.dram_tensor("wu", [128, 9 * 128], F16, kind="ExternalInput")
    wd_d = nc.dram_tensor("wd", [128, 6 * 128], BF16, kind="ExternalInput")
    a2t2_d = nc.dram_tensor("a2t2", [128, ROWS], F16, kind="ExternalInput")
    invbt_d = nc.dram_tensor("invbt", [128, 2 * ROWS], BF16, kind="ExternalInput")
    identb_d = nc.dram_tensor("identb", [128, 128], BF16, kind="ExternalInput")
    outs = nc.dram_tensor("outs", [ROWS, T_], BF16, kind="ExternalOutput")

    PI = math.pi

    with TileContext(nc) as tc:
        with (
            tc.tile_pool(name="const", bufs=1) as pconst,
            tc.tile_pool(name="xut", bufs=3) as pxut,
            tc.tile_pool(name="xat", bufs=3) as pxat,
            tc.tile_pool(name="m", bufs=8) as pm,
            tc.tile_pool(name="s", bufs=2) as ps,
            tc.tile_pool(name="q", bufs=4) as pq,
            tc.tile_pool(name="z", bufs=2) as pz,
            tc.tile_pool(name="og", bufs=2) as pog,
            tc.tile_pool(name="r", bufs=2, space="PSUM") as pr,
            tc.tile_pool(name="pp", bufs=2, space="PSUM") as ppp,
            tc.tile_pool(name="os", bufs=2, space="PSUM") as pos,
        ):
            # ---- consts: DMA then one DVE staging tick each ----
            wu_s = pconst.tile([128, 9 * 128], F16, tag="wu_s")
            nc.sync.dma_start(out=wu_s[:], in_=wu_d[:])
            wd_s = pconst.tile([128, 6 * 128], BF16, tag="wd_s")
            nc.sync.dma_start(out=wd_s[:], in_=wd_d[:])
            a2_s = pconst.tile([128, ROWS], F16, tag="a2_s")
            nc.sync.dma_start(out=a2_s[:], in_=a2t2_d[:])
            ib_s = pconst.tile([128, 2 * ROWS], BF16, tag="ib_s")
            nc.sync.dma_start(out=ib_s[:], in_=invbt_d[:])
            id_s = pconst.tile([128, 128], BF16, tag="id_s")
            nc.sync.dma_start(out=id_s[:], in_=identb_d[:])

            wu = pconst.tile([128, 9 * 128], F16, tag="wu")
            nc.vector.tensor_copy(wu[:], wu_s[:])
            wd = pconst.tile([128, 6 * 128], BF16, tag="wd")
            nc.vector.tensor_copy(wd[:], wd_s[:])
            a2t2 = pconst.tile([128, ROWS], F16, tag="a2t2")
            nc.vector.tensor_copy(a2t2[:], a2_s[:])
            invbt = pconst.tile([128, 2 * ROWS], BF16, tag="invbt")
            nc.vector.tensor_copy(invbt[:], ib_s[:])
            identb = pconst.tile([128, 128], BF16, tag="identb")
            nc.vector.tensor_copy(identb[:], id_s[:])
            mpi = pconst.tile([128, 1], F32, tag="mpi")
            nc.vector.memset(mpi[:], 0.0)

            def wsel(op, b):
                v = 0 if b == 0 else (2 if b == NB - 1 else 1)
                if op in _UOPS:
                    i = _UOPS.index(op) * 3 + v
                    return wu[:, i * 128:(i + 1) * 128]
                i = _DOPS.index(op) * 3 + v
                return wd[:, i * 128:(i + 1) * 128]

            # PE warmup: a transpose that waits on the const staging tick
            warm = pos.tile([128, 4 * 116], BF16, tag="os")
            nc.tensor.transpose(warm[:, 0:128], identb[:], identb[:])

            xuts = {}
            xats = {}
            qs = {}
            ogs = {}

            def stage_in(b):
                s, w, L = block_params(b)
                xut = pxut.tile([128, ROWS], F16, tag="xut")
                nc.sync.dma_start_transpose(xut[:], xs[:, w:w + 128])
                xat2 = pxat.tile([128, ROWS], F16, tag="xat")
                nc.vector.tensor_tensor(out=xat2[:], in0=xut[:], in1=a2t2[:],
                                        op=mybir.AluOpType.mult)
                xuts[b] = xut
                xats[b] = xat2

            r01s = {}
            n2ps = {}

            def stage_upA(b):
                r01 = pr.tile([128, 2 * ROWS], F32, tag="r")
                nc.tensor.matmul(r01[:, 0:ROWS], wsel('U0', b), xats[b][:],
                                 start=True, stop=True)
                nc.tensor.matmul(r01[:, ROWS:], wsel('U1', b), xats[b][:],
                                 start=True, stop=True)
                # n2p = fp16(r + 3072): the fp16 store rounds to an even
                # integer + 3072 (fp16 spacing there is 2.0); the +3072 rides
                # the ACT drain's free bias.
                n2p = pm.tile([128, 2 * ROWS], F16, tag="n2p")
                nc.scalar.activation(n2p[:], r01[:],
                                     mybir.ActivationFunctionType.Copy,
                                     bias=MAGIC16)
                r01s[b] = r01
                n2ps[b] = n2p

            def stage_upB(b):
                # m = (n2p-3072) - r = round2(r) - r in [-1,1];
                # sin(pi*m) = -sin(pi*r); the sign cancels in the square.
                m01 = pm.tile([128, 2 * ROWS], F16, tag="m")
                nc.vector.scalar_tensor_tensor(
                    out=m01[:], in0=n2ps[b][:], scalar=MAGIC16, in1=r01s[b][:],
                    op0=mybir.AluOpType.subtract, op1=mybir.AluOpType.subtract)
                s01 = ps.tile([128, 2 * ROWS], BF16, tag="s")
                nc.scalar.activation(s01[:], m01[:],
                                     mybir.ActivationFunctionType.Sin,
                                     bias=mpi[:], scale=PI)
                sscl = pq.tile([128, 2 * ROWS], BF16, tag="sscl")
                nc.vector.tensor_tensor(out=sscl[:], in0=s01[:], in1=invbt[:],
                                        op=mybir.AluOpType.mult)
                q01 = pq.tile([128, 2 * ROWS], BF16, tag="q")
                nc.vector.tensor_tensor(out=q01[:], in0=s01[:], in1=sscl[:],
                                        op=mybir.AluOpType.mult)
                qs[b] = q01

            def stage_out(b):
                s, w, L = block_params(b)
                g, gi = divmod(b, GRP)
                # out.T accumulated directly in seq-major: for each 128-row
                # chunk c, stationary = xut/q chunk, moving = weight columns.
                P = ppp.tile([128, 4 * 116], F32, tag="p")
                wg, w0, w1 = wsel('G', b), wsel('D0', b), wsel('D1', b)
                for c in range(4):
                    sl = slice(c * 128, (c + 1) * 128)
                    po = P[:, c * 116:c * 116 + L]
                    nc.tensor.matmul(po, xuts[b][:, sl], wg[:, 0:L],
                                     start=True, stop=False)
                    nc.tensor.matmul(po, qs[b][:, sl], w0[:, 0:L],
                                     start=False, stop=False)
                    nc.tensor.matmul(po, qs[b][:, ROWS + c * 128:ROWS + (c + 1) * 128],
                                     w1[:, 0:L], start=False, stop=True)
                if gi == 0:
                    ogs[g] = pog.tile([128, 4 * GRP * 116], BF16, name="og",
                                      tag="og")
                og = ogs[g]
                ogv = og.rearrange("p (c t) -> p c t", c=4)
                nc.scalar.copy(
                    ogv[:, :, gi * 116:gi * 116 + L],
                    P.rearrange("p (c t) -> p c t", c=4)[:, :, 0:L])
                gl = GRP if g < NGRP - 1 else NB - GRP * (NGRP - 1)
                if gi == gl - 1:
                    t0 = 116 * GRP * g
                    tw = (s + L) - t0
                    nc.sync.dma_start(
                        out=outs.rearrange("(c p) t -> p c t", c=4)[:, :, t0:t0 + tw],
                        in_=ogv[:, :, 0:tw])
                    nc.vector.memset(og[0:1, 0:1], 0.0)

            for i in range(NB + 2):
                if i < NB:
                    stage_in(i)
                if 1 <= i <= NB:
                    stage_upA(i - 1)
                if 2 <= i:
                    stage_out(i - 2)
                if 1 <= i <= NB:
                    stage_upB(i - 1)

    nc.compile()
    return nc


_NC_CACHE = {}


def kernel(x, alpha, beta, up_filter, down_filter):
    x = np.asarray(x, dtype=np.float32)
    consts = host_consts(alpha, beta, up_filter, down_filter)

    if 'nc' not in _NC_CACHE:
        _NC_CACHE['nc'] = _build_bass()
    nc = _NC_CACHE['nc']

    rows = x.reshape(B_ * C_, T_)
    in_maps = []
    for k in range(NCORES):
        shard = np.ascontiguousarray(rows[k * ROWS:(k + 1) * ROWS]).astype(np.float16)
        in_maps.append(dict(xs=shard, **consts))

    res = run_bass_kernel_spmd(nc, in_maps, core_ids=list(range(NCORES)),
                               **_RUN_KW)
    out = np.concatenate([np.asarray(r["outs"]).astype(np.float32)
                          for r in res.results], axis=0)
    kernel.last_result = res
    return out.reshape(B_, C_, T_)


_RUN_KW = {}
kernel.last_result = None



# revision 5
# speedup vs baseline: 1.0462x; 1.0462x over previous
"""Trainium2 kernel v3 for nn_Activation1d (BigVGAN up->SnakeBeta->down), 8-core SPMD.

Math (per row; a=exp(alpha), invb=1/(exp(beta)+1e-9) per channel):
    out = G @ x + invb * (D @ sin^2(U @ (a*x)))
where U (2x upsample), D (2x downsample) are banded Toeplitz operators with
replicate-pad boundaries and G = D@U (probed in float64).

Sharding: core k takes channels [k*32, (k+1)*32) x all 16 batches
(rows ordered j = batch*32 + ch'), so the drain's per-partition invb
vector is the same for all four 128-row chunks.

Host preprocessing per core:
    xs_t = x_shard.T           [T, 512] fp16  (G path; natural row loads)
    xa_t = (a*x_shard).T       [T, 512] fp16  (sin path; premult folded here)

On-chip per 116-output block (window w..w+128):
    xut, xat : [128,512] natural DMA loads (contiguous, fast)
    r01  = [U0@xat | U1@xat]                  PSUM fp32   (theta = a*y)
    w01  = add_range_wrap(r01) in [-pi,pi]    DVE custom op, fp16
    s01  = Sin(w01)                           ACT, bf16
    q01  = s01*s01                            DVE/Pool split, bf16
    P1   = G@xut (4 chunks)                   PSUM
    P2   = D0@q0 + D1@q1 (4 chunks)           PSUM
    p1s  = Copy(P1)                           ACT drain, bf16
    og   = (P2 * invb_col) + p1s              DVE STT drain (per-partition invb)
8-block groups of og DMA out as bf16; host casts fp32 + unshards.
"""
import math
import numpy as np
import ml_dtypes

import concourse.bass as bass
import concourse.bacc as bacc
import concourse.mybir as mybir
from concourse.tile import TileContext
from concourse.bass_utils import run_bass_kernel_spmd

F32 = mybir.dt.float32
F16 = mybir.dt.float16
BF16 = mybir.dt.bfloat16

B_, C_, T_ = 16, 256, 8192
K_ = 12
NCORES = 8
CPC = C_ // NCORES                    # 32 channels per core
ROWS = B_ * CPC                       # 512 rows per core
L_ = 116
NB = 71                               # 70 blocks of 116 + tail of 72
GRP = 4
NGRP = (NB + GRP - 1) // GRP          # 9 (last group has 7 blocks)
TP = 512                              # host probe domain
PI = math.pi
SQ_ACT = 332                          # columns of q squared on ACT (rest Pool)


# ---------------------------------------------------------------- host math
def _np_up_linop(x, up_filter):
    n, t = x.shape
    pad = K_ // 2 - 1
    pad_left = pad * 2 + (K_ - 2) // 2
    xp = np.concatenate([np.repeat(x[:, :1], pad, 1), x,
                         np.repeat(x[:, -1:], pad, 1)], axis=1)
    L = t + 2 * pad
    xd = np.zeros((n, 2 * L - 1), dtype=np.float32)
    xd[:, ::2] = xp
    yf = np.zeros((n, 2 * L - 1 + K_ - 1), dtype=np.float32)
    for m in range(K_):
        yf[:, m:m + 2 * L - 1] += np.float32(up_filter[m]) * xd
    return (2.0 * yf[:, pad_left:pad_left + 2 * t]).astype(np.float32)


def _np_down_linop(z, down_filter):
    n, t2 = z.shape
    dpl, dpr = K_ // 2 - 1, K_ // 2
    zp = np.concatenate([np.repeat(z[:, :1], dpl, 1), z,
                         np.repeat(z[:, -1:], dpr, 1)], axis=1)
    t = t2 // 2
    out = np.zeros((n, t), dtype=np.float32)
    for j in range(K_):
        out += np.float32(down_filter[j]) * zp[:, j:j + 2 * t - 1:2]
    return out


def block_params(b):
    """(out_start, window_start, out_len) for block b."""
    if b == NB - 1:
        return 116 * b, T_ - 128, T_ - 116 * b
    return 116 * b, max(0, 116 * b - 6), 116


def _variant_anchor(variant):
    if variant == 'first':
        return 0, 0, 116
    if variant == 'int':
        return 192, 186, 116
    return TP - 72, TP - 128, 72


_VARIANTS = ('first', 'int', 'last')


def build_weights(up_filter, down_filter):
    I = np.eye(TP, dtype=np.float32)
    Ufull = _np_up_linop(I, up_filter).T
    U0p, U1p = Ufull[0::2, :], Ufull[1::2, :]
    I2 = np.eye(2 * TP, dtype=np.float32)
    Dfull = _np_down_linop(I2, down_filter).T
    D0p, D1p = Dfull[:, 0::2], Dfull[:, 1::2]
    Gp = (Dfull.astype(np.float64) @ Ufull.astype(np.float64)).astype(np.float32)

    Ws = {}
    o = np.arange(128)
    p = np.arange(128)
    for variant in _VARIANTS:
        s, w, L = _variant_anchor(variant)
        c_pos = s - 3 + o
        x_pos = w + p
        cv = (c_pos >= 0) & (c_pos < TP)
        xv = (x_pos >= 0) & (x_pos < TP)
        for name, P in (('U0', U0p), ('U1', U1p)):
            W = np.zeros((128, 128), np.float32)
            W[np.ix_(xv, cv)] = P[np.ix_(c_pos[cv], x_pos[xv])].T
            Ws[(name, variant)] = W
        out_pos = s + o
        ov = (o < L) & (out_pos < TP)
        for name, P in (('D0', D0p), ('D1', D1p)):
            W = np.zeros((128, 128), np.float32)
            W[np.ix_(cv, ov)] = P[np.ix_(out_pos[ov], c_pos[cv])].T
            Ws[(name, variant)] = W
        W = np.zeros((128, 128), np.float32)
        W[np.ix_(xv, ov)] = Gp[np.ix_(out_pos[ov], x_pos[xv])].T
        Ws[('G', variant)] = W
    return Ws


def host_weights(up_filter, down_filter):
    Ws = build_weights(np.asarray(up_filter, np.float32),
                       np.asarray(down_filter, np.float32))
    wu = np.concatenate(
        [Ws[(op, v)] for op in ('U0', 'U1') for v in _VARIANTS],
        axis=1).astype(np.float16)                          # [128, 6*128]
    wg = np.concatenate(
        [Ws[('G', v)] for v in _VARIANTS],
        axis=1).astype(np.float16)                          # [128, 3*128]
    wd = np.concatenate(
        [Ws[(op, v)] for op in ('D0', 'D1') for v in _VARIANTS],
        axis=1).astype(ml_dtypes.bfloat16)                  # [128, 6*128]
    return wu, wg, wd


# ---------------------------------------------------------------- bass kernel
def _build_bass():
    nc = bacc.Bacc()
    xin = nc.dram_tensor("xin", [T_, 2 * ROWS], F16, kind="ExternalInput")
    wu_d = nc.dram_tensor("wu", [128, 6 * 128], F16, kind="ExternalInput")
    wg_d = nc.dram_tensor("wg", [128, 3 * 128], F16, kind="ExternalInput")
    wd_d = nc.dram_tensor("wd", [128, 6 * 128], BF16, kind="ExternalInput")
    outs = nc.dram_tensor("outs", [ROWS, T_], BF16, kind="ExternalOutput")

    with TileContext(nc) as tc:
        with (
            tc.tile_pool(name="const", bufs=1) as pconst,
            tc.tile_pool(name="xut", bufs=10) as pxut,
            tc.tile_pool(name="w01", bufs=4) as pw,
            tc.tile_pool(name="s01", bufs=4) as ps,
            tc.tile_pool(name="q01", bufs=4) as pq,
            tc.tile_pool(name="og", bufs=3) as pog,
            tc.tile_pool(name="r", bufs=2, space="PSUM") as pr,
            tc.tile_pool(name="pp", bufs=2, space="PSUM") as ppp,
        ):
            wu = pconst.tile([128, 6 * 128], F16, tag="wu")
            nc.sync.dma_start(out=wu[:], in_=wu_d[:])
            wg = pconst.tile([128, 3 * 128], F16, tag="wg")
            nc.sync.dma_start(out=wg[:], in_=wg_d[:])
            wd = pconst.tile([128, 6 * 128], BF16, tag="wd")
            nc.sync.dma_start(out=wd[:], in_=wd_d[:])

            def vsel(b):
                return 0 if b == 0 else (2 if b == NB - 1 else 1)

            def wsel_u(ph, b):
                i = ph * 3 + vsel(b)
                return wu[:, i * 128:(i + 1) * 128]

            def wsel_g(b):
                i = vsel(b)
                return wg[:, i * 128:(i + 1) * 128]

            def wsel_d(ph, b):
                i = ph * 3 + vsel(b)
                return wd[:, i * 128:(i + 1) * 128]

            xuts, xats, r01s, w01s, s01s, q01s = {}, {}, {}, {}, {}, {}
            pps, ogs = {}, {}

            def st_dma(b):
                s, w, L = block_params(b)
                xt = pxut.tile([128, 2 * ROWS], F16, tag="xt")
                nc.sync.dma_start(out=xt[:], in_=xin[w:w + 128, :])
                xuts[b] = xt

            def st_up(b):
                r01 = pr.tile([128, 2 * ROWS], F32, tag="r")
                nc.tensor.matmul(r01[:, 0:ROWS], wsel_u(0, b),
                                 xuts[b][:, ROWS:], start=True, stop=True)
                nc.tensor.matmul(r01[:, ROWS:], wsel_u(1, b),
                                 xuts[b][:, ROWS:], start=True, stop=True)
                r01s[b] = r01

            def st_wrap(b):
                w01 = pw.tile([128, 2 * ROWS], F16, tag="w01")
                nc.vector.add_range_wrap(w01[:], r01s[b][:],
                                         shift=0.0, bound=PI, period=2 * PI)
                w01s[b] = w01
                del r01s[b]

            def st_sin(b):
                s01 = ps.tile([128, 2 * ROWS], BF16, tag="s01")
                nc.scalar.activation(s01[:], w01s[b][:],
                                     mybir.ActivationFunctionType.Sin)
                s01s[b] = s01
                del w01s[b]

            def st_sq(b):
                s01 = s01s[b]
                q01 = pq.tile([128, 2 * ROWS], BF16, tag="q01")
                nc.scalar.activation(q01[:, 0:SQ_ACT], s01[:, 0:SQ_ACT],
                                     mybir.ActivationFunctionType.Square)
                nc.gpsimd.tensor_tensor(
                    out=q01[:, SQ_ACT:], in0=s01[:, SQ_ACT:],
                    in1=s01[:, SQ_ACT:], op=mybir.AluOpType.mult)
                q01s[b] = q01
                del s01s[b]

            def st_outmm(b):
                s, w, L = block_params(b)
                j, h = divmod(b, 2)
                if h == 0:
                    # [2, 4, 128] halves bank-aligned; chunks at 128-stride so
                    # no matmul dest crosses a 2KB PSUM bank boundary
                    pps[j] = ppp.tile([128, 2 * 512], F32, name="pp",
                                      tag="pp")
                P = pps[j]
                wgb, w0, w1 = wsel_g(b), wsel_d(0, b), wsel_d(1, b)
                q01 = q01s[b]
                for c in range(4):
                    po = P[:, h * 512 + c * 128:h * 512 + c * 128 + L]
                    nc.tensor.matmul(po, xuts[b][:, c * 128:(c + 1) * 128],
                                     wgb[:, 0:L], start=True, stop=False)
                    nc.tensor.matmul(po, q01[:, c * 128:(c + 1) * 128],
                                     w0[:, 0:L], start=False, stop=False)
                    nc.tensor.matmul(
                        po, q01[:, ROWS + c * 128:ROWS + (c + 1) * 128],
                        w1[:, 0:L], start=False, stop=True)
                del xuts[b]
                del q01s[b]

            def st_drain(b):
                # fires at b odd (drains pair b-1,b) or the lone tail block
                s, w, L = block_params(b)
                g, gi = divmod(b, GRP)
                if g not in ogs:
                    ogs[g] = pog.tile([128, 4 * GRP * L_], BF16, name="og",
                                      tag="og")
                og = ogs[g]
                j, h = divmod(b, 2)
                og4 = og.rearrange("p (c g t) -> p c g t", c=4, g=GRP)
                pv4 = pps[j].rearrange("p (u c t) -> p c u t", u=2, c=4)
                if h == 1:
                    nc.vector.tensor_copy(og4[:, :, gi - 1:gi + 1, :],
                                          pv4[:, :, :, 0:L_])
                else:  # lone tail block (b == NB-1 even)
                    nc.vector.tensor_copy(og4[:, :, gi:gi + 1, 0:L],
                                          pv4[:, :, 0:1, 0:L])
                del pps[j]
                gl = GRP if g < NGRP - 1 else NB - GRP * (NGRP - 1)
                if gi == gl - 1:
                    ogv = og.rearrange("p (c t) -> p c t", c=4)
                    t0 = L_ * GRP * g
                    tw = (s + L) - t0
                    nc.sync.dma_start(
                        out=outs.rearrange("(c p) t -> p c t", c=4)[:, :, t0:t0 + tw],
                        in_=ogv[:, :, 0:tw])

            for i in range(NB + 8):
                if i < NB:
                    st_dma(i)
                if 0 <= i - 2 < NB:
                    st_up(i - 2)
                if 0 <= i - 3 < NB:
                    st_wrap(i - 3)
                if 0 <= i - 4 < NB:
                    st_sin(i - 4)
                if 0 <= i - 7 < NB and ((i - 7) % 2 == 1 or i - 7 == NB - 1):
                    st_drain(i - 7)
                if 0 <= i - 5 < NB:
                    st_sq(i - 5)
                if 0 <= i - 6 < NB:
                    st_outmm(i - 6)

    nc.compile()
    return nc


_NC_CACHE = {}


def kernel(x, alpha, beta, up_filter, down_filter):
    x = np.asarray(x, dtype=np.float32)
    alpha = np.asarray(alpha, dtype=np.float32)
    beta = np.asarray(beta, dtype=np.float32)
    a = np.exp(alpha)
    bexp = (np.exp(beta) + 1e-9).astype(np.float32)        # 1/invb
    invb = (1.0 / bexp).astype(np.float32)
    wu, wg, wd = host_weights(up_filter, down_filter)

    if 'nc' not in _NC_CACHE:
        _NC_CACHE['nc'] = _build_bass()
    nc = _NC_CACHE['nc']

    in_maps = []
    for k in range(NCORES):
        ch = slice(k * CPC, (k + 1) * CPC)
        xsh = x[:, ch, :]                                  # [16, 32, T]
        xin = np.empty((T_, 2 * ROWS), dtype=np.float16)
        # cols 0:ROWS -> G path scaled by 1/invb; cols ROWS: -> sin path (a*x)
        xin[:, 0:ROWS] = (xsh * bexp[ch][None, :, None]).reshape(ROWS, T_).T
        xin[:, ROWS:] = (xsh * a[ch][None, :, None]).reshape(ROWS, T_).T
        in_maps.append(dict(xin=xin, wu=wu, wg=wg, wd=wd))

    res = run_bass_kernel_spmd(nc, in_maps, core_ids=list(range(NCORES)),
                               **_RUN_KW)
    out = np.empty((B_, C_, T_), dtype=np.float32)
    for k in range(NCORES):
        ch = slice(k * CPC, (k + 1) * CPC)
        o = np.asarray(res.results[k]["outs"]).astype(np.float32)
        out[:, ch, :] = o.reshape(B_, CPC, T_) * invb[ch][None, :, None]
    kernel.last_result = res
    return out


_RUN_KW = {}
kernel.last_result = None
